# revision 52
# baseline (speedup 1.0000x reference)
"""EMD (Sinkhorn) loss kernel for Trainium2, 8 NeuronCores.

Reference: for each (q, p) pair of a 128x128 grid, run an entropic Sinkhorn
solve on a 32x32 cost matrix; logits[q,p] = sum(flow*sim) * (12.5/32).

Exp-domain formulation (matches the jax log-domain reference):
    K = exp((sim-1)/eps);  v0 = 1
    repeat: r_i = sum_j K_ij v_j ; u = a/r ; s_j = sum_i K_ij u_i ; v = b/s
    logits = sum_ij u_i K_ij v_j sim_ij * (T/32)

Sharding: data-parallel over q (16 q / core -> 2048 independent 32x32
problems per core).

The dominant cost in this environment is the host->device transfer over the
axon tunnel (~70-85 MB/s + ~60ms fixed), so the kernel minimizes shipped
bytes:
  - sim is quantized to uint8; K is dequantized+exponentiated on-device.
  - Only the block [:im_len[q], :s_len[p]] of each 32x32 pair matrix is
    shipped (rows/cols past the length carry marginal weight ~3e-7 and
    contribute O(1e-5) to the logits; a length of 0 means uniform weights,
    so those keep all 32). That's ~25% of the data (~4.8MB). The expansion
    into the fixed on-chip layout is one DMA per pair (the DMA descriptors
    top out at 3-dim patterns, so a whole partition can't be done in one).
  - To keep the SPMD program's DMA patterns compile-time-constant and
    identical across cores, queries are sorted by kept-length and dealt
    round-robin (rank 8k+c -> core c, slot k) padded to the per-slot group
    max, and protos are sorted into 8 groups of 16 padded to the group max.
    The program is specialized to the 16+8 group lengths and cached; a call
    with a different length profile rebuilds it (~1 min, first call only).
  - Marginal weights are built on-device from per-partition length vectors.
  - 50 Sinkhorn iterations (converged to ~6e-3 total error vs the 100-iter
    reference; tolerance is 2e-2).
  - The jitted shard_map executable is cached across calls (no retrace),
    and the donated pre-zeroed output buffers of run_bass_via_pjrt are
    dropped (the program writes every output element).

Wall time is dominated by the axon tunnel's fixed per-call cost (~110ms for
even a no-op jit call in this environment); the 4.8MB payload, device exec
(~10ms), and output fetch pipeline inside it.
"""

import numpy as np

EPS = 0.05
N_ITERS = 65
TEMP = 12.5
Q, P, N1, N2 = 128, 128, 32, 32
N_CORES = 8
QL = Q // N_CORES          # 16 queries per core
PL = 16                    # 16 pairs per partition
FREE = PL * N1 * N2        # 16384
POT = PL * 32              # 512 potential values per partition
QSCALE = 255.0             # uint8 quantization of sim; dequant = (q+0.5)/255


def build_program(n_iters, lqg, lpg):
    from concourse import bacc, tile, mybir

    # total stream bytes per core: sum over partitions of 16*lq*lp
    T = PL * int(np.sum(np.asarray(lqg)[:, None] * np.asarray(lpg)[None, :]))

    nc = bacc.Bacc("TRN2", target_bir_lowering=False, debug=False,
                   enable_asserts=False, num_devices=N_CORES)
    f32 = mybir.dt.float32
    u8 = mybir.dt.uint8
    k8s_d = nc.dram_tensor("k8s", [1, T], u8, kind="ExternalInput")
    lens_d = nc.dram_tensor("lens", [128, 1 + PL], f32, kind="ExternalInput")
    out_d = nc.dram_tensor("out", [128, PL], f32, kind="ExternalOutput")

    with tile.TileContext(nc) as tc:
        _emd_body(tc, n_iters, lqg, lpg, k8s_d, lens_d, out_d)
    nc.compile()
    return nc


def _emd_body(tc, n_iters, lqg, lpg, k8s_d, lens_d, out_d):
    from contextlib import ExitStack
    from concourse import mybir
    import concourse.bass as bass
    nc = tc.nc
    f32 = mybir.dt.float32
    i32 = mybir.dt.int32
    u8 = mybir.dt.uint8
    ADD = mybir.AluOpType.add
    MUL = mybir.AluOpType.mult
    LT = mybir.AluOpType.is_lt
    X = mybir.AxisListType.X
    XY = mybir.AxisListType.XY
    AF = mybir.ActivationFunctionType

    ctx = ExitStack()
    sp = ctx.enter_context(tc.tile_pool(name="sp", bufs=1))

    k8 = sp.tile([128, FREE], u8, name="k8")
    lens = sp.tile_from(lens_d.ap())                # [128, 1+PL]
    lena = lens[:, 0:1]                             # [128, 1]
    lenb = lens[:, 1:1 + PL]                        # [128, PL]
    k = sp.tile([128, FREE], f32, name="k")
    tmp = sp.tile([128, FREE], f32, name="tmp")
    v = sp.tile([128, POT], f32, name="v")
    r = sp.tile([128, POT], f32, name="r")
    ri = sp.tile([128, POT], f32, name="ri")
    u = sp.tile([128, POT], f32, name="u")
    s = sp.tile([128, POT], f32, name="s")
    w = sp.tile([128, POT], f32, name="w")
    outsb = sp.tile([128, PL], f32, name="outsb")
    outsb2 = sp.tile([128, PL], f32, name="outsb2")

    it32 = sp.tile([128, 32], i32, name="it32")
    iotaf = sp.tile([128, 32], f32, name="iotaf")
    wA = sp.tile([128, 32], f32, name="wA")
    rsA = sp.tile([128, 1], f32, name="rsA")
    apre = sp.tile([128, 32], f32, name="apre")
    wB = sp.tile([128, POT], f32, name="wB")
    rsB = sp.tile([128, PL], f32, name="rsB")
    riB = sp.tile([128, PL], f32, name="riB")
    bpre = sp.tile([128, POT], f32, name="bpre")
    biasT = sp.tile([128, 1], f32, name="biasT")

    # ragged load: pair (p, t) <- stream block [lq, lp] scattered into the
    # fixed [16 pairs, 32, 32] layout (rows i >= lq and cols j >= lp stay at
    # the memset value; they carry ~3e-7 marginal weight). The DMA hardware
    # tops out at 3-dim access patterns, so this is one DMA per pair,
    # spread across both hardware-DGE queues (SP and Activation).
    nc.gpsimd.memset(k8[:], 0)
    k8ap = k8[:]
    dap = k8s_d.ap()
    dma_engines = (nc.sync, nc.scalar)
    off = 0
    n_dma = 0
    for p in range(128):
        lq = int(lqg[p // 8])
        lp_ = int(lpg[p % 8])
        base = k8ap[p:p + 1]
        for t in range(PL):
            out_ap = bass.AP(base.tensor, base.offset + t * N1 * N2,
                             [base.ap[0], [N2, lq], [1, lp_]])
            in_ap = bass.AP(dap.tensor, off,
                            [dap.ap[0], [lp_, lq], [1, lp_]])
            dma_engines[n_dma & 1].dma_start(out_ap, in_ap)
            n_dma += 1
            off += lq * lp_

    def v4(t):   # [128, PL, N1, N2] view
        return t[:].rearrange("p (l i j) -> p l i j", i=N1, j=N2)

    def p3(t):   # potential [128, POT] viewed [128, PL, 32]
        return t[:].rearrange("p (l x) -> p l x", x=32)

    def mid_bcast(t):
        # t: [128, (pl, j)] read as [128, pl, i(bcast), j]
        ap = t[:]
        return bass.AP(ap.tensor, ap.offset, [ap.ap[0], [N2, PL], [0, N1], [1, N2]])

    def mid_bcast32(t):
        # t: [128, 32] read as [128, pl(bcast), 32]
        ap = t[:]
        return bass.AP(ap.tensor, ap.offset, [ap.ap[0], [0, PL], [1, 32]])

    def trail_bcast(t):
        # t: [128, (pl, i)] read as [128, (pl, i), j(bcast)]
        return t[:].broadcast_to([128, POT, N2])

    def trail_bcast_pl(t):
        # t: [128, PL] read as [128, PL, 32(bcast)]
        return t[:].broadcast_to([128, PL, 32])

    def v3(t):   # [128, (pl, i), j] view of a big tile
        return t[:].rearrange("p (li j) -> p li j", j=N2)

    def strided_ij(t):
        # big tile [128, (pl, i, j)] read as [128, pl, j, i] (i innermost)
        ap = t[:]
        return bass.AP(ap.tensor, ap.offset,
                       [ap.ap[0], [N1 * N2, PL], [1, N2], [N2, N1]])

    # K = exp((sim - 1)/eps) with sim = (q + 0.5)/255 dequantized on device.
    nc.gpsimd.memset(biasT[:], float((0.5 / QSCALE - 1.0) / EPS))
    nc.scalar.activation(out=k[:], in_=k8[:], func=AF.Exp,
                         scale=float(1.0 / (QSCALE * EPS)),
                         bias=biasT[:])

    # marginal weights from lengths, on device:
    # a = ((iota < lena) + 1e-5) normalized; b likewise per (pl) group
    nc.gpsimd.iota(out=it32[:], pattern=[[1, 32]], base=0, channel_multiplier=0)
    nc.vector.tensor_scalar_add(out=iotaf[:], in0=it32[:], scalar1=0)
    nc.vector.tensor_scalar(out=wA[:], in0=iotaf[:], scalar1=lena[:],
                            scalar2=float(1e-5), op0=LT, op1=ADD)
    nc.vector.tensor_reduce(out=rsA[:], in_=wA[:], axis=X, op=ADD)
    nc.vector.reciprocal(out=rsA[:], in_=rsA[:])
    nc.vector.tensor_scalar(out=apre[:], in0=wA[:], scalar1=rsA[:],
                            scalar2=None, op0=MUL)
    nc.vector.tensor_tensor(out=p3(wB), in0=mid_bcast32(iotaf),
                            in1=trail_bcast_pl(lenb), op=LT)
    nc.vector.tensor_scalar_add(out=wB[:], in0=wB[:], scalar1=float(1e-5))
    nc.vector.tensor_reduce(out=rsB[:], in_=p3(wB), axis=X, op=ADD)
    nc.vector.reciprocal(out=riB[:], in_=rsB[:])
    nc.vector.tensor_tensor(out=p3(bpre), in0=p3(wB),
                            in1=trail_bcast_pl(riB), op=MUL)

    # The 16 pair-slots per partition are independent Sinkhorn chains. Pool
    # can do tensor_tensor but not free-axis tensor_reduce, so the split is
    # by op type: Pool runs the big elementwise multiplies, DVE runs the
    # grouped reduces + reciprocals. Processing the two 8-slot halves as
    # separate chains lets mul(h1) overlap reduce(h0) etc., pipelining the
    # two engines instead of serializing one.
    HP = PL // 2          # 8 pair-slots per half
    HFREE = HP * N1 * N2  # 8192
    HPOT = HP * 32        # 256

    def v4h(t, h):    # [128, 8, 32, 32]
        ap = t[:]
        return bass.AP(ap.tensor, ap.offset + h * HFREE,
                       [ap.ap[0], [N1 * N2, HP], [N2, N1], [1, N2]])

    def p3h(t, h):    # [128, 8, 32]
        ap = t[:]
        return bass.AP(ap.tensor, ap.offset + h * HPOT,
                       [ap.ap[0], [32, HP], [1, 32]])

    def poth(t, h):   # [128, 256] flat potential half
        ap = t[:]
        return bass.AP(ap.tensor, ap.offset + h * HPOT, [ap.ap[0], [1, HPOT]])

    def mid_bh(t, h):     # [128, 8, 32(bcast i), 32]
        ap = t[:]
        return bass.AP(ap.tensor, ap.offset + h * HPOT,
                       [ap.ap[0], [N2, HP], [0, N1], [1, N2]])

    def mid_b32h(t):      # apre [128, 32] -> [128, 8(bcast), 32]
        ap = t[:]
        return bass.AP(ap.tensor, ap.offset, [ap.ap[0], [0, HP], [1, 32]])

    def trail_bh(t, h):   # [128, (8, 32), 32(bcast j)]
        ap = t[:]
        return bass.AP(ap.tensor, ap.offset + h * HPOT,
                       [ap.ap[0], [1, HPOT], [0, N2]])

    def v3h(t, h):    # [128, 256, 32]
        ap = t[:]
        return bass.AP(ap.tensor, ap.offset + h * HFREE,
                       [ap.ap[0], [N2, HPOT], [1, N2]])

    def sij_h(t, h):  # strided [128, 8, 32(j), 32(i)]
        ap = t[:]
        return bass.AP(ap.tensor, ap.offset + h * HFREE,
                       [ap.ap[0], [N1 * N2, HP], [1, N2], [N2, N1]])

    for t in range(n_iters):
        if t == 0:
            nc.vector.tensor_reduce(out=p3h(r, 0), in_=v4h(k, 0), axis=X, op=ADD)
            nc.vector.tensor_reduce(out=p3h(r, 1), in_=v4h(k, 1), axis=X, op=ADD)
        else:
            for h in (0, 1):
                nc.gpsimd.tensor_mul(out=poth(v, h), in0=poth(bpre, h),
                                     in1=poth(w, h))
                nc.gpsimd.tensor_mul(out=v4h(tmp, h), in0=v4h(k, h),
                                     in1=mid_bh(v, h))
            for h in (0, 1):
                nc.vector.tensor_reduce(out=p3h(r, h), in_=v4h(tmp, h),
                                        axis=X, op=ADD)
        # DVE finishes half h's recip+u before reducing half h+1, so Pool can
        # start the column-step multiply of half h early. (Tried
        # reciprocal_approx_fast here: numerically fine but the custom
        # 5-block DVE op costs more than InstReciprocal at this size.)
        for h in (0, 1):
            nc.vector.reciprocal(out=poth(ri, h), in_=poth(r, h))
            nc.vector.tensor_tensor(out=p3h(u, h), in0=mid_b32h(apre),
                                    in1=p3h(ri, h), op=MUL)
            nc.gpsimd.tensor_mul(out=v3h(tmp, h), in0=v3h(k, h),
                                 in1=trail_bh(u, h))
        for h in (0, 1):
            nc.vector.tensor_reduce(out=p3h(s, h), in_=sij_h(tmp, h),
                                    axis=X, op=ADD)
            nc.vector.reciprocal(out=poth(w, h), in_=poth(s, h))

    # final: logits = sum_ij u*K*v*sim with sim = 1 + EPS*ln(K), recomputed
    # on-device. K is dead after the plan product, so Ln runs in-place on the
    # K tile. Split like the loop: Pool multiplies, DVE XY-reduces into
    # disjoint 8-slot halves of the out tiles.
    def o2h(t, h):    # [128, 8] half of a [128, PL] tile
        ap = t[:]
        return bass.AP(ap.tensor, ap.offset + h * HP, [ap.ap[0], [1, HP]])

    for h in (0, 1):
        nc.gpsimd.tensor_mul(out=poth(v, h), in0=poth(bpre, h), in1=poth(w, h))
        nc.gpsimd.tensor_mul(out=v4h(tmp, h), in0=v4h(k, h), in1=mid_bh(v, h))
        nc.gpsimd.tensor_mul(out=v3h(tmp, h), in0=v3h(tmp, h),
                             in1=trail_bh(u, h))
    for h in (0, 1):
        nc.vector.tensor_reduce(out=o2h(outsb, h), in_=v4h(tmp, h),
                                axis=XY, op=ADD)
    nc.scalar.activation(out=k[:], in_=k[:], func=AF.Ln)
    for h in (0, 1):
        nc.gpsimd.tensor_mul(out=v4h(tmp, h), in0=v4h(tmp, h), in1=v4h(k, h))
        nc.vector.tensor_reduce(out=o2h(outsb2, h), in_=v4h(tmp, h),
                                axis=XY, op=ADD)
    nc.vector.tensor_scalar_mul(out=outsb2[:], in0=outsb2[:], scalar1=float(EPS))
    nc.vector.tensor_add(out=outsb[:], in0=outsb[:], in1=outsb2[:])
    nc.vector.tensor_scalar_mul(out=outsb[:], in0=outsb[:], scalar1=float(TEMP / N1))
    nc.sync.dma_start(out_d.ap(), outsb[:])
    ctx.close()


def _make_runner(lqg, lpg):
    """Build the specialized program and a cached jitted shard_map callable."""
    import jax
    from jax.sharding import Mesh, PartitionSpec
    from concourse import mybir
    from concourse import bass2jax
    from concourse.bass2jax import _bass_exec_p, partition_id_tensor

    bass2jax.install_neuronx_cc_hook()

    nc = build_program(N_ITERS, lqg, lpg)
    assert nc.dbg_addr is None

    # Our program writes every element of the output, so the pre-zeroed
    # donated output buffers that run_bass_via_pjrt ships are unnecessary.
    partition_name = nc.partition_id_tensor.name if nc.partition_id_tensor else None
    in_names, out_names, out_avals = [], [], []
    for alloc in nc.m.functions[0].allocations:
        if not isinstance(alloc, mybir.MemoryLocationSet):
            continue
        name = alloc.memorylocations[0].name
        if alloc.kind == "ExternalInput":
            if name != partition_name:
                in_names.append(name)
        elif alloc.kind == "ExternalOutput":
            shape = tuple(alloc.tensor_shape)
            dtype = mybir.dt.np(alloc.dtype)
            out_avals.append(jax.core.ShapedArray(shape, dtype))
            out_names.append(name)
    n_params = len(in_names)
    n_outs = len(out_avals)
    if partition_name is not None:
        in_names.append(partition_name)

    def _body(*args):
        operands = list(args)
        if partition_name is not None:
            operands.append(partition_id_tensor())
        outs = _bass_exec_p.bind(
            *operands,
            out_avals=tuple(out_avals),
            in_names=tuple(in_names),
            out_names=tuple(out_names),
            lowering_input_output_aliases=(),
            sim_require_finite=True,
            sim_require_nnan=True,
            nc=nc,
        )
        return tuple(outs)

    try:
        from jax.experimental.shard_map import shard_map
    except ImportError:
        from jax import shard_map

    devices = jax.devices()[:N_CORES]
    mesh = Mesh(np.asarray(devices), ("core",))
    in_specs = (PartitionSpec("core"),) * n_params
    out_specs = (PartitionSpec("core"),) * n_outs
    sharded = jax.jit(
        shard_map(_body, mesh=mesh, in_specs=in_specs, out_specs=out_specs,
                  check_rep=False),
        keep_unused=True,
    )

    order = {n: i for i, n in enumerate(in_names[:n_params])}
    out_idx = out_names.index("out")

    def run(k8s_g, lens_g, lens_np):
        import time
        args = [None] * n_params
        args[order["k8s"]] = k8s_g
        args[order["lens"]] = lens_g
        # the execution units occasionally wedge transiently
        # (NRT_EXEC_UNIT_UNRECOVERABLE); retry rather than fail the call.
        # Retries use the host copy of lens in case the device-resident
        # buffer was lost with the wedge.
        for attempt in range(3):
            try:
                outs = sharded(*args)
                return np.asarray(outs[out_idx])
            except Exception:
                if attempt == 2:
                    raise
                args[order["lens"]] = lens_np
                time.sleep(2.0)

    return run


_STATES = {}       # (im_len, s_len) bytes -> layout + runner state
_RUNNERS = {}      # (lqg, lpg) -> jitted runner


def _build_state(im_len, s_len):
    lq_eff = np.where(im_len <= 0, 32, np.minimum(im_len, 32)).astype(np.int64)
    lp_eff = np.where(s_len <= 0, 32, np.minimum(s_len, 32)).astype(np.int64)
    qorder = np.argsort(-lq_eff, kind="stable")    # rank -> query id
    porder = np.argsort(-lp_eff, kind="stable")    # rank -> proto id
    lqg = tuple(int(lq_eff[qorder[8 * k]]) for k in range(16))   # q-slot max
    lpg = tuple(int(lp_eff[porder[16 * g]]) for g in range(8))   # p-group max

    key = (lqg, lpg)
    if key not in _RUNNERS:
        _RUNNERS[key] = _make_runner(lqg, lpg)

    # stream offsets per partition (16 pairs of lq*lp each)
    sizes = np.array([PL * lqg[p // 8] * lpg[p % 8] for p in range(128)],
                     np.int64)
    offs = np.concatenate([[0], np.cumsum(sizes)])
    T = int(offs[-1])

    # q(c, k) = qorder[8k + c]
    qmat = qorder.reshape(16, 8)                   # [k, c]
    pgroups = [porder[16 * g:16 * g + 16] for g in range(8)]

    lens_g = np.empty((N_CORES * 128, 1 + PL), np.float32)
    for c in range(N_CORES):
        lens_g[c * 128:(c + 1) * 128, 0] = np.asarray(im_len)[qmat[:, c]].repeat(8)
    lenb_core = np.asarray(s_len)[porder].reshape(8, PL).astype(np.float32)
    lens_g[:, 1:] = np.tile(np.tile(lenb_core, (16, 1)), (N_CORES, 1))

    # lens depends only on the lengths (which key this state), so stage it on
    # device once and reuse the committed array across calls
    import jax
    from jax.sharding import Mesh, PartitionSpec, NamedSharding
    sh = NamedSharding(Mesh(np.asarray(jax.devices()[:N_CORES]), ("core",)),
                       PartitionSpec("core"))
    lens_dev = jax.device_put(lens_g, sh)
    jax.block_until_ready(lens_dev)

    # output scatter: out_g[c*128+p, l] -> logits[q(c,p//8), porder[16*(p%8)+l]]
    rows = np.empty((N_CORES * 128, PL), np.int64)
    cols = np.empty((N_CORES * 128, PL), np.int64)
    for c in range(N_CORES):
        for p in range(128):
            rows[c * 128 + p, :] = qmat[p // 8, c]
            cols[c * 128 + p, :] = pgroups[p % 8]
    flat_idx = (rows * P + cols).ravel()

    return {
        "im_len": np.asarray(im_len).copy(), "s_len": np.asarray(s_len).copy(),
        "run": _RUNNERS[key], "lqg": lqg, "lpg": lpg, "offs": offs, "T": T,
        "qmat": qmat, "pgroups": pgroups, "lens_g": lens_dev, "lens_np": lens_g,
        "flat_idx": flat_idx,
    }


def _pack(sim, st):
    # quantize+pack the ragged stream, one fused gather-multiply-cast per
    # (q-slot, proto-group) across all 8 cores (truncation cast; the +0.5
    # dequant offset is folded into the device-side activation bias).
    # Partition p of core c holds query q(c, p//8) x proto group p%8,
    # block [:lqg, :lpg] per pair. The buffer is reused across calls to
    # avoid first-touch page faults.
    lqg, lpg, offs = st["lqg"], st["lpg"], st["offs"]
    stream = st.get("streambuf")
    if stream is None:
        stream = st["streambuf"] = np.empty((N_CORES, st["T"]), np.uint8)
    qscale = np.float32(QSCALE)
    for k in range(16):
        qcol = st["qmat"][k][:, None]              # [8 cores, 1]
        for g in range(8):
            p = 8 * k + g
            src = sim[qcol, st["pgroups"][g][None, :], :lqg[k], :lpg[g]]
            dst = stream[:, offs[p]:offs[p + 1]].reshape(
                N_CORES, PL, lqg[k], lpg[g])
            np.multiply(src, qscale, out=dst, casting="unsafe")
    return stream


def kernel(similarity_map, im_set, s_seq, im_len, s_len):
    sim = np.asarray(similarity_map, dtype=np.float32)
    im_len = np.asarray(im_len)
    s_len = np.asarray(s_len)

    skey = (im_len.astype(np.int64).tobytes(), s_len.astype(np.int64).tobytes())
    st = _STATES.get(skey)
    if st is None:
        st = _STATES[skey] = _build_state(im_len, s_len)

    stream = _pack(sim, st)
    out_g = st["run"](stream, st["lens_g"], st["lens_np"])    # [1024, 16]
    logits = np.zeros(Q * P, np.float32)
    logits[st["flat_idx"]] = out_g.ravel()
    return logits.reshape(Q, P)


# revision 53
# speedup vs baseline: 1.2112x; 1.2112x over previous
"""EMD (Sinkhorn) loss kernel for Trainium2, 8 NeuronCores.

Reference: for each (q, p) pair of a 128x128 grid, run an entropic Sinkhorn
solve on a 32x32 cost matrix; logits[q,p] = sum(flow*sim) * (12.5/32).

Exp-domain formulation (matches the jax log-domain reference):
    K = exp((sim-1)/eps);  v0 = 1
    repeat: r_i = sum_j K_ij v_j ; u = a/r ; s_j = sum_i K_ij u_i ; v = b/s
    logits = sum_ij u_i K_ij v_j sim_ij * (T/32)

Sharding: data-parallel over q (16 q / core -> 2048 independent 32x32
problems per core).

The dominant cost in this environment is the host->device transfer over the
axon tunnel (~70-85 MB/s + ~60ms fixed), so the kernel minimizes shipped
bytes:
  - sim is quantized to uint8; K is dequantized+exponentiated on-device.
  - Only the block [:im_len[q], :s_len[p]] of each 32x32 pair matrix is
    shipped (rows/cols past the length carry marginal weight ~3e-7 and
    contribute O(1e-5) to the logits; a length of 0 means uniform weights,
    so those keep all 32). That's ~25% of the data (~4.8MB). The expansion
    into the fixed on-chip layout is one DMA per pair (the DMA descriptors
    top out at 3-dim patterns, so a whole partition can't be done in one).
  - To keep the SPMD program's DMA patterns compile-time-constant and
    identical across cores, queries are sorted by kept-length and dealt
    round-robin (rank 8k+c -> core c, slot k) padded to the per-slot group
    max, and protos are sorted into 8 groups of 16 padded to the group max.
    The program is specialized to the 16+8 group lengths and cached; a call
    with a different length profile rebuilds it (~1 min, first call only).
  - Marginal weights are built on-device from per-partition length vectors.
  - 50 Sinkhorn iterations (converged to ~6e-3 total error vs the 100-iter
    reference; tolerance is 2e-2).
  - The jitted shard_map executable is cached across calls (no retrace),
    and the donated pre-zeroed output buffers of run_bass_via_pjrt are
    dropped (the program writes every output element).

Wall time is dominated by the axon tunnel's fixed per-call cost (~110ms for
even a no-op jit call in this environment); the 4.8MB payload, device exec
(~10ms), and output fetch pipeline inside it.
"""

import numpy as np

EPS = 0.05
N_ITERS = 65
TEMP = 12.5
Q, P, N1, N2 = 128, 128, 32, 32
N_CORES = 8
QL = Q // N_CORES          # 16 queries per core
PL = 16                    # 16 pairs per partition
FREE = PL * N1 * N2        # 16384
POT = PL * 32              # 512 potential values per partition
QSCALE = 255.0             # uint8 quantization of sim; dequant = (q+0.5)/255


def build_program(n_iters, lqg, lpg):
    from concourse import bacc, tile, mybir

    # total stream bytes per core: sum over partitions of 16*lq*lp
    T = PL * int(np.sum(np.asarray(lqg)[:, None] * np.asarray(lpg)[None, :]))

    nc = bacc.Bacc("TRN2", target_bir_lowering=False, debug=False,
                   enable_asserts=False, num_devices=N_CORES)
    f32 = mybir.dt.float32
    u8 = mybir.dt.uint8
    k8s_d = nc.dram_tensor("k8s", [1, T], u8, kind="ExternalInput")
    lens_d = nc.dram_tensor("lens", [128, 1 + PL], f32, kind="ExternalInput")
    out_d = nc.dram_tensor("out", [128, PL], f32, kind="ExternalOutput")

    with tile.TileContext(nc) as tc:
        _emd_body(tc, n_iters, lqg, lpg, k8s_d, lens_d, out_d)
    nc.compile()
    return nc


def _emd_body(tc, n_iters, lqg, lpg, k8s_d, lens_d, out_d):
    from contextlib import ExitStack
    from concourse import mybir
    import concourse.bass as bass
    nc = tc.nc
    f32 = mybir.dt.float32
    i32 = mybir.dt.int32
    u8 = mybir.dt.uint8
    ADD = mybir.AluOpType.add
    MUL = mybir.AluOpType.mult
    LT = mybir.AluOpType.is_lt
    X = mybir.AxisListType.X
    XY = mybir.AxisListType.XY
    AF = mybir.ActivationFunctionType

    ctx = ExitStack()
    sp = ctx.enter_context(tc.tile_pool(name="sp", bufs=1))

    k8 = sp.tile([128, FREE], u8, name="k8")
    lens = sp.tile_from(lens_d.ap())                # [128, 1+PL]
    lena = lens[:, 0:1]                             # [128, 1]
    lenb = lens[:, 1:1 + PL]                        # [128, PL]
    k = sp.tile([128, FREE], f32, name="k")
    tmp = sp.tile([128, FREE], f32, name="tmp")
    v = sp.tile([128, POT], f32, name="v")
    r = sp.tile([128, POT], f32, name="r")
    ri = sp.tile([128, POT], f32, name="ri")
    u = sp.tile([128, POT], f32, name="u")
    s = sp.tile([128, POT], f32, name="s")
    w = sp.tile([128, POT], f32, name="w")
    outsb = sp.tile([128, PL], f32, name="outsb")
    outsb2 = sp.tile([128, PL], f32, name="outsb2")

    it32 = sp.tile([128, 32], i32, name="it32")
    iotaf = sp.tile([128, 32], f32, name="iotaf")
    wA = sp.tile([128, 32], f32, name="wA")
    rsA = sp.tile([128, 1], f32, name="rsA")
    apre = sp.tile([128, 32], f32, name="apre")
    wB = sp.tile([128, POT], f32, name="wB")
    rsB = sp.tile([128, PL], f32, name="rsB")
    riB = sp.tile([128, PL], f32, name="riB")
    bpre = sp.tile([128, POT], f32, name="bpre")
    biasT = sp.tile([128, 1], f32, name="biasT")

    # ragged load: pair (p, t) <- stream block [lq, lp] scattered into the
    # fixed [16 pairs, 32, 32] layout (rows i >= lq and cols j >= lp stay at
    # the memset value; they carry ~3e-7 marginal weight). The DMA hardware
    # tops out at 3-dim access patterns, so this is one DMA per pair,
    # spread across both hardware-DGE queues (SP and Activation).
    nc.gpsimd.memset(k8[:], 0)
    k8ap = k8[:]
    dap = k8s_d.ap()
    dma_engines = (nc.sync, nc.scalar)
    off = 0
    n_dma = 0
    for p in range(128):
        lq = int(lqg[p // 8])
        lp_ = int(lpg[p % 8])
        base = k8ap[p:p + 1]
        for t in range(PL):
            out_ap = bass.AP(base.tensor, base.offset + t * N1 * N2,
                             [base.ap[0], [N2, lq], [1, lp_]])
            in_ap = bass.AP(dap.tensor, off,
                            [dap.ap[0], [lp_, lq], [1, lp_]])
            dma_engines[n_dma & 1].dma_start(out_ap, in_ap)
            n_dma += 1
            off += lq * lp_

    def v4(t):   # [128, PL, N1, N2] view
        return t[:].rearrange("p (l i j) -> p l i j", i=N1, j=N2)

    def p3(t):   # potential [128, POT] viewed [128, PL, 32]
        return t[:].rearrange("p (l x) -> p l x", x=32)

    def mid_bcast(t):
        # t: [128, (pl, j)] read as [128, pl, i(bcast), j]
        ap = t[:]
        return bass.AP(ap.tensor, ap.offset, [ap.ap[0], [N2, PL], [0, N1], [1, N2]])

    def mid_bcast32(t):
        # t: [128, 32] read as [128, pl(bcast), 32]
        ap = t[:]
        return bass.AP(ap.tensor, ap.offset, [ap.ap[0], [0, PL], [1, 32]])

    def trail_bcast(t):
        # t: [128, (pl, i)] read as [128, (pl, i), j(bcast)]
        return t[:].broadcast_to([128, POT, N2])

    def trail_bcast_pl(t):
        # t: [128, PL] read as [128, PL, 32(bcast)]
        return t[:].broadcast_to([128, PL, 32])

    def v3(t):   # [128, (pl, i), j] view of a big tile
        return t[:].rearrange("p (li j) -> p li j", j=N2)

    def strided_ij(t):
        # big tile [128, (pl, i, j)] read as [128, pl, j, i] (i innermost)
        ap = t[:]
        return bass.AP(ap.tensor, ap.offset,
                       [ap.ap[0], [N1 * N2, PL], [1, N2], [N2, N1]])

    # K = exp((sim - 1)/eps) with sim = (q + 0.5)/255 dequantized on device.
    nc.gpsimd.memset(biasT[:], float((0.5 / QSCALE - 1.0) / EPS))
    nc.scalar.activation(out=k[:], in_=k8[:], func=AF.Exp,
                         scale=float(1.0 / (QSCALE * EPS)),
                         bias=biasT[:])

    # marginal weights from lengths, on device:
    # a = ((iota < lena) + 1e-5) normalized; b likewise per (pl) group
    nc.gpsimd.iota(out=it32[:], pattern=[[1, 32]], base=0, channel_multiplier=0)
    nc.vector.tensor_scalar_add(out=iotaf[:], in0=it32[:], scalar1=0)
    nc.vector.tensor_scalar(out=wA[:], in0=iotaf[:], scalar1=lena[:],
                            scalar2=float(1e-5), op0=LT, op1=ADD)
    nc.vector.tensor_reduce(out=rsA[:], in_=wA[:], axis=X, op=ADD)
    nc.vector.reciprocal(out=rsA[:], in_=rsA[:])
    nc.vector.tensor_scalar(out=apre[:], in0=wA[:], scalar1=rsA[:],
                            scalar2=None, op0=MUL)
    nc.vector.tensor_tensor(out=p3(wB), in0=mid_bcast32(iotaf),
                            in1=trail_bcast_pl(lenb), op=LT)
    nc.vector.tensor_scalar_add(out=wB[:], in0=wB[:], scalar1=float(1e-5))
    nc.vector.tensor_reduce(out=rsB[:], in_=p3(wB), axis=X, op=ADD)
    nc.vector.reciprocal(out=riB[:], in_=rsB[:])
    nc.vector.tensor_tensor(out=p3(bpre), in0=p3(wB),
                            in1=trail_bcast_pl(riB), op=MUL)

    # The 16 pair-slots per partition are independent Sinkhorn chains. Pool
    # can do tensor_tensor but not free-axis tensor_reduce, so the split is
    # by op type: Pool runs the big elementwise multiplies, DVE runs the
    # grouped reduces + reciprocals. Processing the two 8-slot halves as
    # separate chains lets mul(h1) overlap reduce(h0) etc., pipelining the
    # two engines instead of serializing one.
    HP = PL // 2          # 8 pair-slots per half
    HFREE = HP * N1 * N2  # 8192
    HPOT = HP * 32        # 256

    def v4h(t, h):    # [128, 8, 32, 32]
        ap = t[:]
        return bass.AP(ap.tensor, ap.offset + h * HFREE,
                       [ap.ap[0], [N1 * N2, HP], [N2, N1], [1, N2]])

    def p3h(t, h):    # [128, 8, 32]
        ap = t[:]
        return bass.AP(ap.tensor, ap.offset + h * HPOT,
                       [ap.ap[0], [32, HP], [1, 32]])

    def poth(t, h):   # [128, 256] flat potential half
        ap = t[:]
        return bass.AP(ap.tensor, ap.offset + h * HPOT, [ap.ap[0], [1, HPOT]])

    def mid_bh(t, h):     # [128, 8, 32(bcast i), 32]
        ap = t[:]
        return bass.AP(ap.tensor, ap.offset + h * HPOT,
                       [ap.ap[0], [N2, HP], [0, N1], [1, N2]])

    def mid_b32h(t):      # apre [128, 32] -> [128, 8(bcast), 32]
        ap = t[:]
        return bass.AP(ap.tensor, ap.offset, [ap.ap[0], [0, HP], [1, 32]])

    def trail_bh(t, h):   # [128, (8, 32), 32(bcast j)]
        ap = t[:]
        return bass.AP(ap.tensor, ap.offset + h * HPOT,
                       [ap.ap[0], [1, HPOT], [0, N2]])

    def v3h(t, h):    # [128, 256, 32]
        ap = t[:]
        return bass.AP(ap.tensor, ap.offset + h * HFREE,
                       [ap.ap[0], [N2, HPOT], [1, N2]])

    def sij_h(t, h):  # strided [128, 8, 32(j), 32(i)]
        ap = t[:]
        return bass.AP(ap.tensor, ap.offset + h * HFREE,
                       [ap.ap[0], [N1 * N2, HP], [1, N2], [N2, N1]])

    for t in range(n_iters):
        if t == 0:
            nc.vector.tensor_reduce(out=p3h(r, 0), in_=v4h(k, 0), axis=X, op=ADD)
            nc.vector.tensor_reduce(out=p3h(r, 1), in_=v4h(k, 1), axis=X, op=ADD)
        else:
            for h in (0, 1):
                nc.gpsimd.tensor_mul(out=poth(v, h), in0=poth(bpre, h),
                                     in1=poth(w, h))
                nc.gpsimd.tensor_mul(out=v4h(tmp, h), in0=v4h(k, h),
                                     in1=mid_bh(v, h))
            for h in (0, 1):
                nc.vector.tensor_reduce(out=p3h(r, h), in_=v4h(tmp, h),
                                        axis=X, op=ADD)
        # DVE finishes half h's recip+u before reducing half h+1, so Pool can
        # start the column-step multiply of half h early. (Measured in the
        # cost model: reciprocal_approx_fast is cost-identical to
        # InstReciprocal at this size, and moving u to Pool saves only
        # ~29us/call — both below noise, so the simpler form stays.)
        for h in (0, 1):
            nc.vector.reciprocal(out=poth(ri, h), in_=poth(r, h))
            nc.vector.tensor_tensor(out=p3h(u, h), in0=mid_b32h(apre),
                                    in1=p3h(ri, h), op=MUL)
            nc.gpsimd.tensor_mul(out=v3h(tmp, h), in0=v3h(k, h),
                                 in1=trail_bh(u, h))
        for h in (0, 1):
            nc.vector.tensor_reduce(out=p3h(s, h), in_=sij_h(tmp, h),
                                    axis=X, op=ADD)
            nc.vector.reciprocal(out=poth(w, h), in_=poth(s, h))

    # final: logits = sum_ij u*K*v*sim with sim = 1 + EPS*ln(K), recomputed
    # on-device. K is dead after the plan product, so Ln runs in-place on the
    # K tile. Split like the loop: Pool multiplies, DVE XY-reduces into
    # disjoint 8-slot halves of the out tiles.
    def o2h(t, h):    # [128, 8] half of a [128, PL] tile
        ap = t[:]
        return bass.AP(ap.tensor, ap.offset + h * HP, [ap.ap[0], [1, HP]])

    for h in (0, 1):
        nc.gpsimd.tensor_mul(out=poth(v, h), in0=poth(bpre, h), in1=poth(w, h))
        nc.gpsimd.tensor_mul(out=v4h(tmp, h), in0=v4h(k, h), in1=mid_bh(v, h))
        nc.gpsimd.tensor_mul(out=v3h(tmp, h), in0=v3h(tmp, h),
                             in1=trail_bh(u, h))
    for h in (0, 1):
        nc.vector.tensor_reduce(out=o2h(outsb, h), in_=v4h(tmp, h),
                                axis=XY, op=ADD)
    nc.scalar.activation(out=k[:], in_=k[:], func=AF.Ln)
    for h in (0, 1):
        nc.gpsimd.tensor_mul(out=v4h(tmp, h), in0=v4h(tmp, h), in1=v4h(k, h))
        nc.vector.tensor_reduce(out=o2h(outsb2, h), in_=v4h(tmp, h),
                                axis=XY, op=ADD)
    nc.vector.tensor_scalar_mul(out=outsb2[:], in0=outsb2[:], scalar1=float(EPS))
    nc.vector.tensor_add(out=outsb[:], in0=outsb[:], in1=outsb2[:])
    nc.vector.tensor_scalar_mul(out=outsb[:], in0=outsb[:], scalar1=float(TEMP / N1))
    nc.sync.dma_start(out_d.ap(), outsb[:])
    ctx.close()


def _make_runner(lqg, lpg):
    """Build the specialized program and a cached jitted shard_map callable."""
    import jax
    from jax.sharding import Mesh, PartitionSpec
    from concourse import mybir
    from concourse import bass2jax
    from concourse.bass2jax import _bass_exec_p, partition_id_tensor

    bass2jax.install_neuronx_cc_hook()

    nc = build_program(N_ITERS, lqg, lpg)
    assert nc.dbg_addr is None

    # Our program writes every element of the output, so the pre-zeroed
    # donated output buffers that run_bass_via_pjrt ships are unnecessary.
    partition_name = nc.partition_id_tensor.name if nc.partition_id_tensor else None
    in_names, out_names, out_avals = [], [], []
    for alloc in nc.m.functions[0].allocations:
        if not isinstance(alloc, mybir.MemoryLocationSet):
            continue
        name = alloc.memorylocations[0].name
        if alloc.kind == "ExternalInput":
            if name != partition_name:
                in_names.append(name)
        elif alloc.kind == "ExternalOutput":
            shape = tuple(alloc.tensor_shape)
            dtype = mybir.dt.np(alloc.dtype)
            out_avals.append(jax.core.ShapedArray(shape, dtype))
            out_names.append(name)
    n_params = len(in_names)
    n_outs = len(out_avals)
    if partition_name is not None:
        in_names.append(partition_name)

    def _body(*args):
        operands = list(args)
        if partition_name is not None:
            operands.append(partition_id_tensor())
        outs = _bass_exec_p.bind(
            *operands,
            out_avals=tuple(out_avals),
            in_names=tuple(in_names),
            out_names=tuple(out_names),
            lowering_input_output_aliases=(),
            sim_require_finite=True,
            sim_require_nnan=True,
            nc=nc,
        )
        return tuple(outs)

    try:
        from jax.experimental.shard_map import shard_map
    except ImportError:
        from jax import shard_map

    devices = jax.devices()[:N_CORES]
    mesh = Mesh(np.asarray(devices), ("core",))
    in_specs = (PartitionSpec("core"),) * n_params
    out_specs = (PartitionSpec("core"),) * n_outs
    sharded = jax.jit(
        shard_map(_body, mesh=mesh, in_specs=in_specs, out_specs=out_specs,
                  check_rep=False),
        keep_unused=True,
    )

    order = {n: i for i, n in enumerate(in_names[:n_params])}
    out_idx = out_names.index("out")

    def run(k8s_g, lens_g, lens_np):
        import time
        args = [None] * n_params
        args[order["k8s"]] = k8s_g
        args[order["lens"]] = lens_g
        # the execution units occasionally wedge transiently
        # (NRT_EXEC_UNIT_UNRECOVERABLE); retry rather than fail the call.
        # Retries use the host copy of lens in case the device-resident
        # buffer was lost with the wedge.
        for attempt in range(3):
            try:
                outs = sharded(*args)
                return np.asarray(outs[out_idx])
            except Exception:
                if attempt == 2:
                    raise
                args[order["lens"]] = lens_np
                time.sleep(2.0)

    return run


_STATES = {}       # (im_len, s_len) bytes -> layout + runner state
_RUNNERS = {}      # (lqg, lpg) -> jitted runner


def _build_state(im_len, s_len):
    lq_eff = np.where(im_len <= 0, 32, np.minimum(im_len, 32)).astype(np.int64)
    lp_eff = np.where(s_len <= 0, 32, np.minimum(s_len, 32)).astype(np.int64)
    qorder = np.argsort(-lq_eff, kind="stable")    # rank -> query id
    porder = np.argsort(-lp_eff, kind="stable")    # rank -> proto id
    lqg = tuple(int(lq_eff[qorder[8 * k]]) for k in range(16))   # q-slot max
    lpg = tuple(int(lp_eff[porder[16 * g]]) for g in range(8))   # p-group max

    key = (lqg, lpg)
    if key not in _RUNNERS:
        _RUNNERS[key] = _make_runner(lqg, lpg)

    # stream offsets per partition (16 pairs of lq*lp each)
    sizes = np.array([PL * lqg[p // 8] * lpg[p % 8] for p in range(128)],
                     np.int64)
    offs = np.concatenate([[0], np.cumsum(sizes)])
    T = int(offs[-1])

    # q(c, k) = qorder[8k + c]
    qmat = qorder.reshape(16, 8)                   # [k, c]
    pgroups = [porder[16 * g:16 * g + 16] for g in range(8)]

    lens_g = np.empty((N_CORES * 128, 1 + PL), np.float32)
    for c in range(N_CORES):
        lens_g[c * 128:(c + 1) * 128, 0] = np.asarray(im_len)[qmat[:, c]].repeat(8)
    lenb_core = np.asarray(s_len)[porder].reshape(8, PL).astype(np.float32)
    lens_g[:, 1:] = np.tile(np.tile(lenb_core, (16, 1)), (N_CORES, 1))

    # lens depends only on the lengths (which key this state), so stage it on
    # device once and reuse the committed array across calls
    import jax
    from jax.sharding import Mesh, PartitionSpec, NamedSharding
    sh = NamedSharding(Mesh(np.asarray(jax.devices()[:N_CORES]), ("core",)),
                       PartitionSpec("core"))
    lens_dev = jax.device_put(lens_g, sh)
    jax.block_until_ready(lens_dev)

    # output scatter: out_g[c*128+p, l] -> logits[q(c,p//8), porder[16*(p%8)+l]]
    rows = np.empty((N_CORES * 128, PL), np.int64)
    cols = np.empty((N_CORES * 128, PL), np.int64)
    for c in range(N_CORES):
        for p in range(128):
            rows[c * 128 + p, :] = qmat[p // 8, c]
            cols[c * 128 + p, :] = pgroups[p % 8]
    flat_idx = (rows * P + cols).ravel()

    return {
        "im_len": np.asarray(im_len).copy(), "s_len": np.asarray(s_len).copy(),
        "run": _RUNNERS[key], "lqg": lqg, "lpg": lpg, "offs": offs, "T": T,
        "qmat": qmat, "pgroups": pgroups, "lens_g": lens_dev, "lens_np": lens_g,
        "flat_idx": flat_idx,
    }


def _pack(sim, st):
    # quantize+pack the ragged stream, one fused gather-multiply-cast per
    # (q-slot, proto-group) across all 8 cores (truncation cast; the +0.5
    # dequant offset is folded into the device-side activation bias).
    # Partition p of core c holds query q(c, p//8) x proto group p%8,
    # block [:lqg, :lpg] per pair. The buffer is reused across calls to
    # avoid first-touch page faults.
    lqg, lpg, offs = st["lqg"], st["lpg"], st["offs"]
    stream = st.get("streambuf")
    if stream is None:
        stream = st["streambuf"] = np.empty((N_CORES, st["T"]), np.uint8)
    qscale = np.float32(QSCALE)
    for k in range(16):
        qcol = st["qmat"][k][:, None]              # [8 cores, 1]
        for g in range(8):
            p = 8 * k + g
            src = sim[qcol, st["pgroups"][g][None, :], :lqg[k], :lpg[g]]
            dst = stream[:, offs[p]:offs[p + 1]].reshape(
                N_CORES, PL, lqg[k], lpg[g])
            np.multiply(src, qscale, out=dst, casting="unsafe")
    return stream


def kernel(similarity_map, im_set, s_seq, im_len, s_len):
    sim = np.asarray(similarity_map, dtype=np.float32)
    im_len = np.asarray(im_len)
    s_len = np.asarray(s_len)

    skey = (im_len.astype(np.int64).tobytes(), s_len.astype(np.int64).tobytes())
    st = _STATES.get(skey)
    if st is None:
        st = _STATES[skey] = _build_state(im_len, s_len)

    stream = _pack(sim, st)
    out_g = st["run"](stream, st["lens_g"], st["lens_np"])    # [1024, 16]
    logits = np.zeros(Q * P, np.float32)
    logits[st["flat_idx"]] = out_g.ravel()
    return logits.reshape(Q, P)


# revision 54
# speedup vs baseline: 1.2808x; 1.0575x over previous
"""EMD (Sinkhorn) loss kernel for Trainium2, 8 NeuronCores.

Reference: for each (q, p) pair of a 128x128 grid, run an entropic Sinkhorn
solve on a 32x32 cost matrix; logits[q,p] = sum(flow*sim) * (12.5/32).

Exp-domain formulation (matches the jax log-domain reference):
    K = exp((sim-1)/eps);  v0 = 1
    repeat: r_i = sum_j K_ij v_j ; u = a/r ; s_j = sum_i K_ij u_i ; v = b/s
    logits = sum_ij u_i K_ij v_j sim_ij * (T/32)

Sharding: data-parallel over q (16 q / core -> 2048 independent 32x32
problems per core).

The dominant cost in this environment is the host->device transfer over the
axon tunnel (~70-85 MB/s + ~60ms fixed), so the kernel minimizes shipped
bytes:
  - sim is quantized to uint8; K is dequantized+exponentiated on-device.
  - Only the block [:im_len[q], :s_len[p]] of each 32x32 pair matrix is
    shipped (rows/cols past the length carry marginal weight ~3e-7 and
    contribute O(1e-5) to the logits; a length of 0 means uniform weights,
    so those keep all 32). That's ~25% of the data (~4.8MB). The expansion
    into the fixed on-chip layout is one DMA per pair (the DMA descriptors
    top out at 3-dim patterns, so a whole partition can't be done in one).
  - To keep the SPMD program's DMA patterns compile-time-constant and
    identical across cores, queries are sorted by kept-length and dealt
    round-robin (rank 8k+c -> core c, slot k) padded to the per-slot group
    max, and protos are sorted into 8 groups of 16 padded to the group max.
    The program is specialized to the 16+8 group lengths and cached; a call
    with a different length profile rebuilds it (~1 min, first call only).
  - Marginal weights are built on-device from per-partition length vectors.
  - 50 Sinkhorn iterations (converged to ~6e-3 total error vs the 100-iter
    reference; tolerance is 2e-2).
  - The jitted shard_map executable is cached across calls (no retrace),
    and the donated pre-zeroed output buffers of run_bass_via_pjrt are
    dropped (the program writes every output element).

Wall time is dominated by the axon tunnel's fixed per-call cost (~110ms for
even a no-op jit call in this environment); the 4.8MB payload, device exec
(~10ms), and output fetch pipeline inside it.
"""

import numpy as np

EPS = 0.05
N_ITERS = 65
TEMP = 12.5
Q, P, N1, N2 = 128, 128, 32, 32
N_CORES = 8
QL = Q // N_CORES          # 16 queries per core
PL = 16                    # 16 pairs per partition
FREE = PL * N1 * N2        # 16384
POT = PL * 32              # 512 potential values per partition
QSCALE = 255.0             # uint8 quantization of sim; dequant = (q+0.5)/255


def build_program(n_iters, lqg, lpg):
    from concourse import bacc, tile, mybir

    # total stream bytes per core: sum over partitions of 16*lq*lp
    T = PL * int(np.sum(np.asarray(lqg)[:, None] * np.asarray(lpg)[None, :]))

    nc = bacc.Bacc("TRN2", target_bir_lowering=False, debug=False,
                   enable_asserts=False, num_devices=N_CORES)
    f32 = mybir.dt.float32
    u8 = mybir.dt.uint8
    k8s_d = nc.dram_tensor("k8s", [1, T], u8, kind="ExternalInput")
    lens_d = nc.dram_tensor("lens", [128, 1 + PL], f32, kind="ExternalInput")
    out_d = nc.dram_tensor("out", [128, PL], f32, kind="ExternalOutput")

    with tile.TileContext(nc) as tc:
        _emd_body(tc, n_iters, lqg, lpg, k8s_d, lens_d, out_d)
    nc.compile()
    return nc


def _emd_body(tc, n_iters, lqg, lpg, k8s_d, lens_d, out_d):
    from contextlib import ExitStack
    from concourse import mybir
    import concourse.bass as bass
    nc = tc.nc
    f32 = mybir.dt.float32
    i32 = mybir.dt.int32
    u8 = mybir.dt.uint8
    ADD = mybir.AluOpType.add
    MUL = mybir.AluOpType.mult
    LT = mybir.AluOpType.is_lt
    X = mybir.AxisListType.X
    XY = mybir.AxisListType.XY
    AF = mybir.ActivationFunctionType

    ctx = ExitStack()
    sp = ctx.enter_context(tc.tile_pool(name="sp", bufs=1))

    k8 = sp.tile([128, FREE], u8, name="k8")
    lens = sp.tile_from(lens_d.ap())                # [128, 1+PL]
    lena = lens[:, 0:1]                             # [128, 1]
    lenb = lens[:, 1:1 + PL]                        # [128, PL]
    k = sp.tile([128, FREE], f32, name="k")
    tmp = sp.tile([128, FREE], f32, name="tmp")
    v = sp.tile([128, POT], f32, name="v")
    r = sp.tile([128, POT], f32, name="r")
    ri = sp.tile([128, POT], f32, name="ri")
    u = sp.tile([128, POT], f32, name="u")
    s = sp.tile([128, POT], f32, name="s")
    w = sp.tile([128, POT], f32, name="w")
    outsb = sp.tile([128, PL], f32, name="outsb")
    outsb2 = sp.tile([128, PL], f32, name="outsb2")

    it32 = sp.tile([128, 32], i32, name="it32")
    iotaf = sp.tile([128, 32], f32, name="iotaf")
    wA = sp.tile([128, 32], f32, name="wA")
    rsA = sp.tile([128, 1], f32, name="rsA")
    apre = sp.tile([128, 32], f32, name="apre")
    wB = sp.tile([128, POT], f32, name="wB")
    rsB = sp.tile([128, PL], f32, name="rsB")
    riB = sp.tile([128, PL], f32, name="riB")
    bpre = sp.tile([128, POT], f32, name="bpre")
    biasT = sp.tile([128, 1], f32, name="biasT")

    # ragged load: pair (p, t) <- stream block [lq, lp] scattered into the
    # fixed [16 pairs, 32, 32] layout (rows i >= lq and cols j >= lp stay at
    # the memset value; they carry ~3e-7 marginal weight). The DMA hardware
    # tops out at 3-dim access patterns, so this is one DMA per pair,
    # spread across both hardware-DGE queues (SP and Activation).
    nc.gpsimd.memset(k8[:], 0)
    k8ap = k8[:]
    dap = k8s_d.ap()
    dma_engines = (nc.sync, nc.scalar)
    off = 0
    n_dma = 0
    for p in range(128):
        lq = int(lqg[p // 8])
        lp_ = int(lpg[p % 8])
        base = k8ap[p:p + 1]
        for t in range(PL):
            out_ap = bass.AP(base.tensor, base.offset + t * N1 * N2,
                             [base.ap[0], [N2, lq], [1, lp_]])
            in_ap = bass.AP(dap.tensor, off,
                            [dap.ap[0], [lp_, lq], [1, lp_]])
            dma_engines[n_dma & 1].dma_start(out_ap, in_ap)
            n_dma += 1
            off += lq * lp_

    def v4(t):   # [128, PL, N1, N2] view
        return t[:].rearrange("p (l i j) -> p l i j", i=N1, j=N2)

    def p3(t):   # potential [128, POT] viewed [128, PL, 32]
        return t[:].rearrange("p (l x) -> p l x", x=32)

    def mid_bcast(t):
        # t: [128, (pl, j)] read as [128, pl, i(bcast), j]
        ap = t[:]
        return bass.AP(ap.tensor, ap.offset, [ap.ap[0], [N2, PL], [0, N1], [1, N2]])

    def mid_bcast32(t):
        # t: [128, 32] read as [128, pl(bcast), 32]
        ap = t[:]
        return bass.AP(ap.tensor, ap.offset, [ap.ap[0], [0, PL], [1, 32]])

    def trail_bcast(t):
        # t: [128, (pl, i)] read as [128, (pl, i), j(bcast)]
        return t[:].broadcast_to([128, POT, N2])

    def trail_bcast_pl(t):
        # t: [128, PL] read as [128, PL, 32(bcast)]
        return t[:].broadcast_to([128, PL, 32])

    def v3(t):   # [128, (pl, i), j] view of a big tile
        return t[:].rearrange("p (li j) -> p li j", j=N2)

    def strided_ij(t):
        # big tile [128, (pl, i, j)] read as [128, pl, j, i] (i innermost)
        ap = t[:]
        return bass.AP(ap.tensor, ap.offset,
                       [ap.ap[0], [N1 * N2, PL], [1, N2], [N2, N1]])

    # K = exp((sim - 1)/eps) with sim = (q + 0.5)/255 dequantized on device.
    nc.gpsimd.memset(biasT[:], float((0.5 / QSCALE - 1.0) / EPS))
    nc.scalar.activation(out=k[:], in_=k8[:], func=AF.Exp,
                         scale=float(1.0 / (QSCALE * EPS)),
                         bias=biasT[:])

    # marginal weights from lengths, on device:
    # a = ((iota < lena) + 1e-5) normalized; b likewise per (pl) group
    nc.gpsimd.iota(out=it32[:], pattern=[[1, 32]], base=0, channel_multiplier=0)
    nc.vector.tensor_scalar_add(out=iotaf[:], in0=it32[:], scalar1=0)
    nc.vector.tensor_scalar(out=wA[:], in0=iotaf[:], scalar1=lena[:],
                            scalar2=float(1e-5), op0=LT, op1=ADD)
    nc.vector.tensor_reduce(out=rsA[:], in_=wA[:], axis=X, op=ADD)
    nc.vector.reciprocal(out=rsA[:], in_=rsA[:])
    nc.vector.tensor_scalar(out=apre[:], in0=wA[:], scalar1=rsA[:],
                            scalar2=None, op0=MUL)
    nc.vector.tensor_tensor(out=p3(wB), in0=mid_bcast32(iotaf),
                            in1=trail_bcast_pl(lenb), op=LT)
    nc.vector.tensor_scalar_add(out=wB[:], in0=wB[:], scalar1=float(1e-5))
    nc.vector.tensor_reduce(out=rsB[:], in_=p3(wB), axis=X, op=ADD)
    nc.vector.reciprocal(out=riB[:], in_=rsB[:])
    nc.vector.tensor_tensor(out=p3(bpre), in0=p3(wB),
                            in1=trail_bcast_pl(riB), op=MUL)

    # The 16 pair-slots per partition are independent Sinkhorn chains. Pool
    # can do tensor_tensor but not free-axis tensor_reduce, so the split is
    # by op type: Pool runs the big elementwise multiplies, DVE runs the
    # grouped reduces + reciprocals. Processing the two 8-slot halves as
    # separate chains lets mul(h1) overlap reduce(h0) etc., pipelining the
    # two engines instead of serializing one.
    HP = PL // 2          # 8 pair-slots per half
    HFREE = HP * N1 * N2  # 8192
    HPOT = HP * 32        # 256

    def v4h(t, h):    # [128, 8, 32, 32]
        ap = t[:]
        return bass.AP(ap.tensor, ap.offset + h * HFREE,
                       [ap.ap[0], [N1 * N2, HP], [N2, N1], [1, N2]])

    def p3h(t, h):    # [128, 8, 32]
        ap = t[:]
        return bass.AP(ap.tensor, ap.offset + h * HPOT,
                       [ap.ap[0], [32, HP], [1, 32]])

    def poth(t, h):   # [128, 256] flat potential half
        ap = t[:]
        return bass.AP(ap.tensor, ap.offset + h * HPOT, [ap.ap[0], [1, HPOT]])

    def mid_bh(t, h):     # [128, 8, 32(bcast i), 32]
        ap = t[:]
        return bass.AP(ap.tensor, ap.offset + h * HPOT,
                       [ap.ap[0], [N2, HP], [0, N1], [1, N2]])

    def mid_b32h(t):      # apre [128, 32] -> [128, 8(bcast), 32]
        ap = t[:]
        return bass.AP(ap.tensor, ap.offset, [ap.ap[0], [0, HP], [1, 32]])

    def trail_bh(t, h):   # [128, (8, 32), 32(bcast j)]
        ap = t[:]
        return bass.AP(ap.tensor, ap.offset + h * HPOT,
                       [ap.ap[0], [1, HPOT], [0, N2]])

    def v3h(t, h):    # [128, 256, 32]
        ap = t[:]
        return bass.AP(ap.tensor, ap.offset + h * HFREE,
                       [ap.ap[0], [N2, HPOT], [1, N2]])

    def sij_h(t, h):  # strided [128, 8, 32(j), 32(i)]
        ap = t[:]
        return bass.AP(ap.tensor, ap.offset + h * HFREE,
                       [ap.ap[0], [N1 * N2, HP], [1, N2], [N2, N1]])

    for t in range(n_iters):
        if t == 0:
            nc.vector.tensor_reduce(out=p3h(r, 0), in_=v4h(k, 0), axis=X, op=ADD)
            nc.vector.tensor_reduce(out=p3h(r, 1), in_=v4h(k, 1), axis=X, op=ADD)
        else:
            for h in (0, 1):
                nc.gpsimd.tensor_mul(out=poth(v, h), in0=poth(bpre, h),
                                     in1=poth(w, h))
                nc.gpsimd.tensor_mul(out=v4h(tmp, h), in0=v4h(k, h),
                                     in1=mid_bh(v, h))
            for h in (0, 1):
                nc.vector.tensor_reduce(out=p3h(r, h), in_=v4h(tmp, h),
                                        axis=X, op=ADD)
        # DVE finishes half h's recip+u before reducing half h+1, so Pool can
        # start the column-step multiply of half h early. (Measured in the
        # cost model: reciprocal_approx_fast is cost-identical to
        # InstReciprocal at this size, and moving u to Pool saves only
        # ~29us/call — both below noise, so the simpler form stays.)
        for h in (0, 1):
            nc.vector.reciprocal(out=poth(ri, h), in_=poth(r, h))
            nc.vector.tensor_tensor(out=p3h(u, h), in0=mid_b32h(apre),
                                    in1=p3h(ri, h), op=MUL)
            nc.gpsimd.tensor_mul(out=v3h(tmp, h), in0=v3h(k, h),
                                 in1=trail_bh(u, h))
        for h in (0, 1):
            nc.vector.tensor_reduce(out=p3h(s, h), in_=sij_h(tmp, h),
                                    axis=X, op=ADD)
            nc.vector.reciprocal(out=poth(w, h), in_=poth(s, h))

    # final: logits = sum_ij u*K*v*sim with sim = 1 + EPS*ln(K), recomputed
    # on-device. K is dead after the plan product, so Ln runs in-place on the
    # K tile. Split like the loop: Pool multiplies, DVE XY-reduces into
    # disjoint 8-slot halves of the out tiles.
    def o2h(t, h):    # [128, 8] half of a [128, PL] tile
        ap = t[:]
        return bass.AP(ap.tensor, ap.offset + h * HP, [ap.ap[0], [1, HP]])

    for h in (0, 1):
        nc.gpsimd.tensor_mul(out=poth(v, h), in0=poth(bpre, h), in1=poth(w, h))
        nc.gpsimd.tensor_mul(out=v4h(tmp, h), in0=v4h(k, h), in1=mid_bh(v, h))
        nc.gpsimd.tensor_mul(out=v3h(tmp, h), in0=v3h(tmp, h),
                             in1=trail_bh(u, h))
    for h in (0, 1):
        nc.vector.tensor_reduce(out=o2h(outsb, h), in_=v4h(tmp, h),
                                axis=XY, op=ADD)
    nc.scalar.activation(out=k[:], in_=k[:], func=AF.Ln)
    for h in (0, 1):
        nc.gpsimd.tensor_mul(out=v4h(tmp, h), in0=v4h(tmp, h), in1=v4h(k, h))
        nc.vector.tensor_reduce(out=o2h(outsb2, h), in_=v4h(tmp, h),
                                axis=XY, op=ADD)
    nc.vector.tensor_scalar_mul(out=outsb2[:], in0=outsb2[:], scalar1=float(EPS))
    nc.vector.tensor_add(out=outsb[:], in0=outsb[:], in1=outsb2[:])
    nc.vector.tensor_scalar_mul(out=outsb[:], in0=outsb[:], scalar1=float(TEMP / N1))
    nc.sync.dma_start(out_d.ap(), outsb[:])
    ctx.close()


def _make_runner(lqg, lpg):
    """Build the specialized program and a cached jitted shard_map callable."""
    import jax
    from jax.sharding import Mesh, PartitionSpec
    from concourse import mybir
    from concourse import bass2jax
    from concourse.bass2jax import _bass_exec_p, partition_id_tensor

    bass2jax.install_neuronx_cc_hook()

    nc = build_program(N_ITERS, lqg, lpg)
    assert nc.dbg_addr is None

    # Our program writes every element of the output, so the pre-zeroed
    # donated output buffers that run_bass_via_pjrt ships are unnecessary.
    partition_name = nc.partition_id_tensor.name if nc.partition_id_tensor else None
    in_names, out_names, out_avals = [], [], []
    for alloc in nc.m.functions[0].allocations:
        if not isinstance(alloc, mybir.MemoryLocationSet):
            continue
        name = alloc.memorylocations[0].name
        if alloc.kind == "ExternalInput":
            if name != partition_name:
                in_names.append(name)
        elif alloc.kind == "ExternalOutput":
            shape = tuple(alloc.tensor_shape)
            dtype = mybir.dt.np(alloc.dtype)
            out_avals.append(jax.core.ShapedArray(shape, dtype))
            out_names.append(name)
    n_params = len(in_names)
    n_outs = len(out_avals)
    if partition_name is not None:
        in_names.append(partition_name)

    def _body(*args):
        operands = list(args)
        if partition_name is not None:
            operands.append(partition_id_tensor())
        outs = _bass_exec_p.bind(
            *operands,
            out_avals=tuple(out_avals),
            in_names=tuple(in_names),
            out_names=tuple(out_names),
            lowering_input_output_aliases=(),
            sim_require_finite=True,
            sim_require_nnan=True,
            nc=nc,
        )
        return tuple(outs)

    try:
        from jax.experimental.shard_map import shard_map
    except ImportError:
        from jax import shard_map

    devices = jax.devices()[:N_CORES]
    mesh = Mesh(np.asarray(devices), ("core",))
    in_specs = (PartitionSpec("core"),) * n_params
    out_specs = (PartitionSpec("core"),) * n_outs
    sharded = jax.jit(
        shard_map(_body, mesh=mesh, in_specs=in_specs, out_specs=out_specs,
                  check_rep=False),
        keep_unused=True,
    )

    order = {n: i for i, n in enumerate(in_names[:n_params])}
    out_idx = out_names.index("out")

    def run(k8s_g, lens_g, lens_np):
        import time
        args = [None] * n_params
        args[order["k8s"]] = k8s_g
        args[order["lens"]] = lens_g
        # the execution units occasionally wedge transiently
        # (NRT_EXEC_UNIT_UNRECOVERABLE), sometimes for several seconds;
        # retry with backoff rather than fail the call. Retries use the
        # host copy of lens in case the device-resident buffer was lost
        # with the wedge.
        for attempt, delay in enumerate((2.0, 4.0, 8.0, 0.0)):
            try:
                outs = sharded(*args)
                return np.asarray(outs[out_idx])
            except Exception:
                if attempt == 3:
                    raise
                args[order["lens"]] = lens_np
                time.sleep(delay)

    return run


_STATES = {}       # (im_len, s_len) bytes -> layout + runner state
_RUNNERS = {}      # (lqg, lpg) -> jitted runner


def _build_state(im_len, s_len):
    lq_eff = np.where(im_len <= 0, 32, np.minimum(im_len, 32)).astype(np.int64)
    lp_eff = np.where(s_len <= 0, 32, np.minimum(s_len, 32)).astype(np.int64)
    qorder = np.argsort(-lq_eff, kind="stable")    # rank -> query id
    porder = np.argsort(-lp_eff, kind="stable")    # rank -> proto id
    lqg = tuple(int(lq_eff[qorder[8 * k]]) for k in range(16))   # q-slot max
    lpg = tuple(int(lp_eff[porder[16 * g]]) for g in range(8))   # p-group max

    key = (lqg, lpg)
    if key not in _RUNNERS:
        _RUNNERS[key] = _make_runner(lqg, lpg)

    # stream offsets per partition (16 pairs of lq*lp each)
    sizes = np.array([PL * lqg[p // 8] * lpg[p % 8] for p in range(128)],
                     np.int64)
    offs = np.concatenate([[0], np.cumsum(sizes)])
    T = int(offs[-1])

    # q(c, k) = qorder[8k + c]
    qmat = qorder.reshape(16, 8)                   # [k, c]
    pgroups = [porder[16 * g:16 * g + 16] for g in range(8)]

    lens_g = np.empty((N_CORES * 128, 1 + PL), np.float32)
    for c in range(N_CORES):
        lens_g[c * 128:(c + 1) * 128, 0] = np.asarray(im_len)[qmat[:, c]].repeat(8)
    lenb_core = np.asarray(s_len)[porder].reshape(8, PL).astype(np.float32)
    lens_g[:, 1:] = np.tile(np.tile(lenb_core, (16, 1)), (N_CORES, 1))

    # lens depends only on the lengths (which key this state), so stage it on
    # device once and reuse the committed array across calls
    import jax
    from jax.sharding import Mesh, PartitionSpec, NamedSharding
    sh = NamedSharding(Mesh(np.asarray(jax.devices()[:N_CORES]), ("core",)),
                       PartitionSpec("core"))
    lens_dev = jax.device_put(lens_g, sh)
    jax.block_until_ready(lens_dev)

    # output scatter: out_g[c*128+p, l] -> logits[q(c,p//8), porder[16*(p%8)+l]]
    rows = np.empty((N_CORES * 128, PL), np.int64)
    cols = np.empty((N_CORES * 128, PL), np.int64)
    for c in range(N_CORES):
        for p in range(128):
            rows[c * 128 + p, :] = qmat[p // 8, c]
            cols[c * 128 + p, :] = pgroups[p % 8]
    flat_idx = (rows * P + cols).ravel()

    return {
        "im_len": np.asarray(im_len).copy(), "s_len": np.asarray(s_len).copy(),
        "run": _RUNNERS[key], "lqg": lqg, "lpg": lpg, "offs": offs, "T": T,
        "qmat": qmat, "pgroups": pgroups, "lens_g": lens_dev, "lens_np": lens_g,
        "flat_idx": flat_idx,
    }


def _pack(sim, st):
    # quantize+pack the ragged stream, one fused gather-multiply-cast per
    # (q-slot, proto-group) across all 8 cores (truncation cast; the +0.5
    # dequant offset is folded into the device-side activation bias).
    # Partition p of core c holds query q(c, p//8) x proto group p%8,
    # block [:lqg, :lpg] per pair. The buffer is reused across calls to
    # avoid first-touch page faults.
    lqg, lpg, offs = st["lqg"], st["lpg"], st["offs"]
    stream = st.get("streambuf")
    if stream is None:
        stream = st["streambuf"] = np.empty((N_CORES, st["T"]), np.uint8)
    qscale = np.float32(QSCALE)
    for k in range(16):
        qcol = st["qmat"][k][:, None]              # [8 cores, 1]
        for g in range(8):
            p = 8 * k + g
            src = sim[qcol, st["pgroups"][g][None, :], :lqg[k], :lpg[g]]
            dst = stream[:, offs[p]:offs[p + 1]].reshape(
                N_CORES, PL, lqg[k], lpg[g])
            np.multiply(src, qscale, out=dst, casting="unsafe")
    return stream


def kernel(similarity_map, im_set, s_seq, im_len, s_len):
    sim = np.asarray(similarity_map, dtype=np.float32)
    im_len = np.asarray(im_len)
    s_len = np.asarray(s_len)

    skey = (im_len.astype(np.int64).tobytes(), s_len.astype(np.int64).tobytes())
    st = _STATES.get(skey)
    if st is None:
        st = _STATES[skey] = _build_state(im_len, s_len)

    stream = _pack(sim, st)
    out_g = st["run"](stream, st["lens_g"], st["lens_np"])    # [1024, 16]
    logits = np.zeros(Q * P, np.float32)
    logits[st["flat_idx"]] = out_g.ravel()
    return logits.reshape(Q, P)
